# revision 17
# baseline (speedup 1.0000x reference)
"""Multi-head attention (B=4, S=2048, D=1024, H=16) on 8 Trainium2 NeuronCores.

Sharding: data-parallel over the 4 batches x tensor-parallel over head halves
(2 groups of 8 heads).  core c -> batch c//2, heads (c%2)*8 .. (c%2)*8+7.
Each core computes a partial output x[b] attention over its 8 heads projected
through its slice of w_out; the host sums the two partials per batch (+b_out).

Key design points vs the v1 kernel:
  * x is transposed and cast to fp16 on the HOST -> no PE transposes, no cold
    HAM phase; 16 junk warm-up matmuls flip the PE to 2.4GHz during input DMA.
  * exp(scores) is computed two ways, alternating by s_k chunk:
      even j: ACT exp (exact)   -> DVE mask multiply
      odd  j: DVE Schraudolph   -> DVE mask multiply
    Schraudolph: u = int16(scores*A + B); the bit pattern of u IS
    fp16(exp(scores/8) * 2^-3 * r), r a +-3% sawtooth that cancels through
    the softmax normalization.  B is tuned so both paths share the 2^-3 scale.
    This halves the ACT engine load (the v1 bottleneck: 271us of serial exp).
  * scores are computed in 512-wide s_q blocks so PSUM holds double-buffered
    scores (2x2 banks) + 4 ctx accumulators (4 banks).
  * v is augmented with a ones column -> AV also produces the softmax
    denominator (M=65 matmuls).  Normalization is fused into the PSUM->SBUF
    ctx copy via scalar_tensor_tensor.
  * all inputs fp16 (host cast), mask as fp16 0/1, output fp16 partials.
"""

import math
import sys

from contextlib import ExitStack

import numpy as np

if "/opt/trn_rl_repo" not in sys.path:
    sys.path.insert(0, "/opt/trn_rl_repo")

B, S, D, H = 4, 2048, 1024, 16
DH = 64          # head dim
HPC = 8          # heads per core
CD = HPC * DH    # 512 cols per core per q/k/v
NCORES = 8

SK_CHUNKS = 16       # s_k chunks of 128
NQB = 4              # s_q blocks of 512
NPAIR = 4            # head pairs per core
NBLK = 4             # s-blocks of 512 for projection phase

# Schraudolph constants: u = scores*A + B; int16 bits of u read as fp16 give
# exp(scores/8) * 2^(B/1024-15) * r(frac),  mean(r) folded into B so the
# effective scale matches the ACT path's exp(s/8 - 3*ln2) = exp(s/8)*2^-3.
SCH_A = 1024.0 * math.log2(math.e) / 8.0          # 184.664
SCH_B = 12288.0 - 1024.0 * math.log2(1.0406)      # ~12229.2
ACT_BIAS = -3.0 * math.log(2.0)                   # -2.0794

# engine load-balance: which s_k chunks take the DVE Schraudolph exp, and
# which ACT chunks get their mask multiply on GPSIMD instead of DVE
# engine load-balance: which s_k chunks take the DVE Schraudolph exp (the
# rest use ACT exp), and which ACT chunks get their mask multiply on GPSIMD
# instead of DVE.  GPSIMD's ~2.5us op latency would stall the in-order PE
# queue, so GPSIMD-masked chunks are the first NDEFER js and their AV
# matmuls are emitted NDEFER+2 chunks late (PSUM accumulation order is free).
DVE_CHUNKS = frozenset({6, 9, 12, 15})
GPS_MASK_CHUNKS = frozenset({0, 1, 2, 3, 4})
NDEFER = 5


def _build():
    import concourse.bass as bass
    import concourse.mybir as mybir
    import concourse.tile as tile
    from concourse import bacc
    from concourse.bass import ds

    f32 = mybir.dt.float32
    f16 = mybir.dt.float16
    i16 = mybir.dt.int16
    Alu = mybir.AluOpType
    Act = mybir.ActivationFunctionType

    nc = bacc.Bacc(name="mha8v2")

    xT_d = nc.dram_tensor("xT", [D, S], f16, kind="ExternalInput")
    mask_d = nc.dram_tensor("mask", [S, S], f16, kind="ExternalInput")
    wq_d = nc.dram_tensor("wq", [D, CD], f16, kind="ExternalInput")
    wk_d = nc.dram_tensor("wk", [D, CD], f16, kind="ExternalInput")
    wv_d = nc.dram_tensor("wv", [D, CD], f16, kind="ExternalInput")
    bq_d = nc.dram_tensor("bq", [CD], f32, kind="ExternalInput")
    bk_d = nc.dram_tensor("bk", [CD], f32, kind="ExternalInput")
    bv_d = nc.dram_tensor("bv", [CD], f32, kind="ExternalInput")
    wo_d = nc.dram_tensor("wo", [CD, D], f16, kind="ExternalInput")
    out_d = nc.dram_tensor("out", [S, D], f16, kind="ExternalOutput")

    with tile.TileContext(nc) as tc, ExitStack() as top:
        const = top.enter_context(tc.tile_pool(name="const", bufs=1))

        # HAM warm-up fodder: the PE clock-gate opens after ~3.4us of matmul
        # activity; burn junk matmuls while the input DMAs stream.
        warm_sb = const.tile([128, 512], f16)
        nc.vector.memset(warm_sb, 0.0)

        exp_bias = const.tile([128, 1], f32)
        nc.vector.memset(exp_bias, ACT_BIAS)

        # biases for q/k: [128, 4] -> column m*128+p holds b[m*128+p]
        bq_sb = const.tile([128, NBLK], f32)
        bk_sb = const.tile([128, NBLK], f32)
        nc.sync.dma_start(out=bq_sb, in_=bq_d.rearrange("(m p) -> p m", p=128))
        nc.sync.dma_start(out=bk_sb, in_=bk_d.rearrange("(m p) -> p m", p=128))

        # persistent activation tensors
        qk_pool = top.enter_context(tc.tile_pool(name="qk", bufs=1))
        qT_sb = qk_pool.tile([128, NPAIR, S], f16)   # q^T, head pair-major
        kT_sb = qk_pool.tile([128, NPAIR, S], f16)
        v_pool = top.enter_context(tc.tile_pool(name="vpool", bufs=1))
        v_sb = v_pool.tile([128, SK_CHUNKS, HPC, 66], f16)  # [.., 0:64]=v, 64=ones
        ctx_pool = top.enter_context(tc.tile_pool(name="ctxp", bufs=1))
        ctxT_sb = ctx_pool.tile([128, NPAIR, S], f16)
        mask_pool = top.enter_context(tc.tile_pool(name="maskp", bufs=1))
        mk_sb = mask_pool.tile([128, SK_CHUNKS, S], f16)    # mask[s_k, s_q] 0/1

        nc.vector.memset(v_sb[:, :, :, 64:65], 1.0)

        # ---------------- phase 1: QKV projection -------------------------
        with ExitStack() as ph1:
            wpool = ph1.enter_context(tc.tile_pool(name="wpool", bufs=1))
            xt_pool = ph1.enter_context(tc.tile_pool(name="xt", bufs=1))
            warm_ps_pool = ph1.enter_context(
                tc.tile_pool(name="warmps", bufs=1, space="PSUM")
            )
            prj_ps_pool = ph1.enter_context(
                tc.tile_pool(name="prjps", bufs=4, space="PSUM")
            )

            warm_ps = warm_ps_pool.tile([128, 512], f32)
            for _ in range(16):
                nc.tensor.matmul(
                    warm_ps, lhsT=warm_sb[:, 0:128], rhs=warm_sb,
                    start=True, stop=True,
                )

            # DMA order matters: wq + first xT half gate the first matmuls.
            wq_sb = wpool.tile([128, 8, CD], f16)
            wk_sb = wpool.tile([128, 8, CD], f16)
            wv_sb = wpool.tile([128, 8, CD], f16)
            xT = xt_pool.tile([128, 8, S], f16)
            nc.sync.dma_start(out=wq_sb, in_=wq_d.rearrange("(k p) n -> p k n", p=128))
            nc.sync.dma_start(
                out=xT[:, :, 0:1024],
                in_=xT_d.rearrange("(k p) s -> p k s", p=128)[:, :, 0:1024],
            )
            nc.sync.dma_start(
                out=xT[:, :, 1024:2048],
                in_=xT_d.rearrange("(k p) s -> p k s", p=128)[:, :, 1024:2048],
            )
            nc.sync.dma_start(out=wk_sb, in_=wk_d.rearrange("(k p) n -> p k n", p=128))
            nc.sync.dma_start(out=wv_sb, in_=wv_d.rearrange("(k p) n -> p k n", p=128))

            bv_row = wpool.tile([1, CD], f32)
            nc.sync.dma_start(out=bv_row, in_=bv_d[None, :])
            bv_bc = wpool.tile([128, CD], f32)
            nc.gpsimd.partition_broadcast(bv_bc, bv_row)

            # mask DMAs on the gpsimd queue; needed only at attention start
            for j in range(SK_CHUNKS):
                nc.gpsimd.dma_start(
                    out=mk_sb[:, j, :], in_=mask_d[ds(j * 128, 128), :]
                )

            # qT / kT: [128 (pair-local dh), pair, S]
            for which, w_sb, b_sb, dst in (
                ("q", wq_sb, bq_sb, qT_sb),
                ("k", wk_sb, bk_sb, kT_sb),
            ):
                for m in range(4):
                    for n in range(NBLK):
                        pps = prj_ps_pool.tile([128, 512], f32, tag="pps")
                        for k in range(8):
                            nc.tensor.matmul(
                                pps,
                                lhsT=w_sb[:, k, ds(m * 128, 128)],
                                rhs=xT[:, k, ds(n * 512, 512)],
                                start=(k == 0),
                                stop=(k == 7),
                            )
                        nc.vector.tensor_scalar_add(
                            out=dst[:, m, ds(n * 512, 512)],
                            in0=pps,
                            scalar1=b_sb[:, ds(m, 1)],
                        )

            # v (natural layout + bias; ones column preset)
            for m16 in range(16):
                vps = prj_ps_pool.tile([128, 512], f32, tag="pps")
                for k in range(8):
                    nc.tensor.matmul(
                        vps,
                        lhsT=xT[:, k, ds(m16 * 128, 128)],
                        rhs=wv_sb[:, k, :],
                        start=(k == 0),
                        stop=(k == 7),
                    )
                nc.vector.tensor_tensor(
                    out=v_sb[:, m16, :, 0:64],
                    in0=vps.rearrange("p (h e) -> p h e", h=HPC),
                    in1=bv_bc.rearrange("p (h e) -> p h e", h=HPC),
                    op=Alu.add,
                )

        # ---------------- phase 2: attention ------------------------------
        with ExitStack() as ph2:
            expm_pool = ph2.enter_context(tc.tile_pool(name="expm", bufs=10))
            raw_pool = ph2.enter_context(tc.tile_pool(name="rawp", bufs=4))
            sc_ps_pool = ph2.enter_context(
                tc.tile_pool(name="scps", bufs=2, space="PSUM")
            )
            ctx_ps_pool = ph2.enter_context(
                tc.tile_pool(name="ctxps", bufs=4, space="PSUM")
            )
            rc_pool = ph2.enter_context(tc.tile_pool(name="rcp", bufs=3))

            for qb in range(NQB):          # 512-wide s_q block
                q0 = qb * 512
                for c in range(NPAIR):     # head pair
                    ctx_ps = [
                        ctx_ps_pool.tile(
                            [128, 512], f32, tag="ctxps", name=f"ctxps{a}"
                        )
                        for a in range(2)
                    ]

                    def emit_qk(j):
                        sc = sc_ps_pool.tile([128, 2, 512], f32, tag="scps")
                        # a=0/a=1 hit disjoint PE row groups (base partitions
                        # 0/64) and run concurrently
                        for a in range(2):
                            nc.tensor.matmul(
                                sc[:, a, :],
                                lhsT=kT_sb[ds(a * 64, 64), c, ds(j * 128, 128)],
                                rhs=qT_sb[ds(a * 64, 64), c, ds(q0, 512)],
                                start=True,
                                stop=True,
                            )
                        return sc

                    def emit_ew(j, sc):
                        mslice = mk_sb[:, j, None, ds(q0, 512)].to_broadcast(
                            (128, 2, 512)
                        )
                        expm = expm_pool.tile([128, 2, 512], i16, tag="expm")
                        if j in DVE_CHUNKS:
                            # DVE path: Schraudolph int16 exp, then mask
                            u = raw_pool.tile([128, 2, 512], i16, tag="raw")
                            nc.vector.tensor_scalar(
                                out=u,
                                in0=sc,
                                scalar1=SCH_A,
                                scalar2=SCH_B,
                                op0=Alu.mult,
                                op1=Alu.add,
                            )
                            nc.vector.tensor_tensor(
                                out=expm,
                                in0=u,
                                in1=mslice,
                                op=Alu.mult,
                            )
                        else:
                            # ACT path: exact exp(s/8 - 3ln2), then mask on
                            # DVE or GPSIMD (load-balanced)
                            raw = raw_pool.tile([128, 2, 512], i16, tag="raw")
                            nc.scalar.activation(
                                out=raw.bitcast(f16),
                                in_=sc,
                                func=Act.Exp,
                                scale=0.125,
                                bias=exp_bias,
                            )
                            eng = nc.gpsimd if j in GPS_MASK_CHUNKS else nc.vector
                            eng.tensor_tensor(
                                out=expm.bitcast(f16),
                                in0=raw.bitcast(f16),
                                in1=mslice,
                                op=Alu.mult,
                            )
                        return expm

                    # emission order: AV(NDEFER) first; AV(jd<NDEFER) rides at
                    # jp=jd+NDEFER+2; AV(15) is always emitted last
                    first_av = NDEFER
                    last_av = SK_CHUNKS - 1

                    def emit_av(j, expm):
                        for a in range(2):
                            nc.tensor.matmul(
                                ctx_ps[a][0:65, :],
                                lhsT=v_sb[:, j, c * 2 + a, 0:65],
                                rhs=expm.bitcast(f16)[:, a, :],
                                start=(j == first_av),
                                stop=(j == last_av),
                            )

                    expms = {}
                    pending = None
                    for j in range(SK_CHUNKS):
                        sc = emit_qk(j)
                        if pending is not None:
                            jp = pending[0]
                            expms[jp] = emit_ew(*pending)
                            if jp >= NDEFER:
                                emit_av(jp, expms.pop(jp))
                            # deferred AV of chunk jp-NDEFER-2 rides here
                            jd = jp - NDEFER - 2
                            if 0 <= jd < NDEFER:
                                emit_av(jd, expms.pop(jd))
                        pending = (j, sc)
                    jp = pending[0]
                    expms[jp] = emit_ew(*pending)
                    emit_av(jp, expms.pop(jp))
                    for jd in sorted(expms):
                        emit_av(jd, expms.pop(jd))

                    # normalization: den rows live at psum partition 64;
                    # ACT-copy to SBUF, DVE approx-reciprocal, then GPSIMD
                    # broadcast + fused scale-on-copy (slack via ctx bufs=4).
                    den2 = rc_pool.tile([1, 2, 512], f32, tag="den")
                    for a in range(2):
                        nc.scalar.copy(
                            out=den2[0:1, a, :], in_=ctx_ps[a][64:65, :]
                        )
                    denr = rc_pool.tile([1, 2, 512], f32, tag="denr")
                    nc.vector.reciprocal_approx_fast(out=denr, in_=den2)
                    rbc = rc_pool.tile([128, 2, 512], f32, tag="rbc")
                    nc.gpsimd.partition_broadcast(rbc, denr)
                    for a in range(2):
                        nc.vector.scalar_tensor_tensor(
                            out=ctxT_sb[ds(a * 64, 64), c, ds(q0, 512)],
                            in0=ctx_ps[a][0:64, :],
                            scalar=1.0,
                            in1=rbc[ds(a * 64, 64), a, :],
                            op0=Alu.mult,
                            op1=Alu.mult,
                        )

        # ---------------- phase 3: output projection -----------------------
        with ExitStack() as ph3:
            out_ps_pool = ph3.enter_context(
                tc.tile_pool(name="outps", bufs=3, space="PSUM")
            )
            ost_pool = ph3.enter_context(tc.tile_pool(name="ost", bufs=3))
            ph3_const = ph3.enter_context(tc.tile_pool(name="ph3c", bufs=1))

            wo_sb = ph3_const.tile([128, 4, D], f16)
            nc.sync.dma_start(
                out=wo_sb, in_=wo_d.rearrange("(r p) n -> p r n", p=128)
            )
            for m in range(16):
                ops = out_ps_pool.tile([128, D], f32, tag="ops")
                for r in range(4):
                    for n2 in range(2):
                        nc.tensor.matmul(
                            ops[:, ds(n2 * 512, 512)],
                            lhsT=ctxT_sb[:, r, ds(m * 128, 128)],
                            rhs=wo_sb[:, r, ds(n2 * 512, 512)],
                            start=(r == 0),
                            stop=(r == 3),
                        )
                ost = ost_pool.tile([128, D], f16, tag="ost")
                nc.scalar.copy(out=ost, in_=ops)
                nc.sync.dma_start(out=out_d[ds(m * 128, 128), :], in_=ost)

    nc.compile()
    return nc


_NC = None


def _get_nc():
    global _NC
    if _NC is None:
        _NC = _build()
    return _NC


def make_in_maps(inputs):
    x = np.asarray(inputs["x"], dtype=np.float32)
    mask = np.asarray(inputs["mask"])
    w_qkv = np.asarray(inputs["w_qkv"], dtype=np.float32)
    b_qkv = np.asarray(inputs["b_qkv"], dtype=np.float32)
    w_out = np.asarray(inputs["w_out"], dtype=np.float32)

    w16 = w_qkv.astype(np.float16)
    wo16 = w_out.astype(np.float16)

    in_maps = []
    for core in range(NCORES):
        b = core // 2
        h0 = (core % 2) * CD
        in_maps.append(
            {
                "xT": np.ascontiguousarray(x[b].astype(np.float16).T),
                # device wants mask[s_k, s_q] as fp16 0/1
                "mask": np.ascontiguousarray(
                    mask[b, 0].T.astype(np.float16)
                ),
                "wq": np.ascontiguousarray(w16[:, h0 : h0 + CD]),
                "wk": np.ascontiguousarray(w16[:, D + h0 : D + h0 + CD]),
                "wv": np.ascontiguousarray(w16[:, 2 * D + h0 : 2 * D + h0 + CD]),
                "bq": np.ascontiguousarray(b_qkv[h0 : h0 + CD]),
                "bk": np.ascontiguousarray(b_qkv[D + h0 : D + h0 + CD]),
                "bv": np.ascontiguousarray(b_qkv[2 * D + h0 : 2 * D + h0 + CD]),
                "wo": np.ascontiguousarray(wo16[h0 : h0 + CD, :]),
            }
        )
    return in_maps


def gather_out(core_outs, b_out):
    return np.stack(
        [
            core_outs[2 * b].astype(np.float32)
            + core_outs[2 * b + 1].astype(np.float32)
            + b_out
            for b in range(B)
        ],
        axis=0,
    )


def run(inputs, trace=False):
    """Returns (output, BassKernelResults)."""
    from concourse import bass_utils

    nc = _get_nc()
    in_maps = make_in_maps(inputs)
    res = bass_utils.run_bass_kernel_spmd(
        nc, in_maps, core_ids=list(range(NCORES)), trace=trace
    )
    b_out = np.asarray(inputs["b_out"], dtype=np.float32)
    out = gather_out([r["out"] for r in res.results], b_out)
    return out, res


def kernel(**inputs) -> np.ndarray:
    out, _ = run(inputs, trace=False)
    return out


# revision 18
# speedup vs baseline: 1.3000x; 1.3000x over previous
"""Multi-head attention (B=4, S=2048, D=1024, H=16) on 8 Trainium2 NeuronCores.

Sharding: data-parallel over the 4 batches x tensor-parallel over head halves
(2 groups of 8 heads).  core c -> batch c//2, heads (c%2)*8 .. (c%2)*8+7.
Each core computes a partial output x[b] attention over its 8 heads projected
through its slice of w_out; the host sums the two partials per batch (+b_out).

Key design points vs the v1 kernel:
  * x is transposed and cast to fp16 on the HOST -> no PE transposes, no cold
    HAM phase; 16 junk warm-up matmuls flip the PE to 2.4GHz during input DMA.
  * exp(scores) is computed two ways, alternating by s_k chunk:
      even j: ACT exp (exact)   -> DVE mask multiply
      odd  j: DVE Schraudolph   -> DVE mask multiply
    Schraudolph: u = int16(scores*A + B); the bit pattern of u IS
    fp16(exp(scores/8) * 2^-3 * r), r a +-3% sawtooth that cancels through
    the softmax normalization.  B is tuned so both paths share the 2^-3 scale.
    This halves the ACT engine load (the v1 bottleneck: 271us of serial exp).
  * scores are computed in 512-wide s_q blocks so PSUM holds double-buffered
    scores (2x2 banks) + 4 ctx accumulators (4 banks).
  * v is augmented with a ones column -> AV also produces the softmax
    denominator (M=65 matmuls).  Normalization is fused into the PSUM->SBUF
    ctx copy via scalar_tensor_tensor.
  * all inputs fp16 (host cast), mask as fp16 0/1, output fp16 partials.
"""

import math
import sys

from contextlib import ExitStack

import numpy as np

if "/opt/trn_rl_repo" not in sys.path:
    sys.path.insert(0, "/opt/trn_rl_repo")

B, S, D, H = 4, 2048, 1024, 16
DH = 64          # head dim
HPC = 8          # heads per core
CD = HPC * DH    # 512 cols per core per q/k/v
NCORES = 8

SK_CHUNKS = 16       # s_k chunks of 128
NQB = 4              # s_q blocks of 512
NPAIR = 4            # head pairs per core
NBLK = 4             # s-blocks of 512 for projection phase

# Schraudolph constants: u = scores*A + B; int16 bits of u read as fp16 give
# exp(scores/8) * 2^(B/1024-15) * r(frac),  mean(r) folded into B so the
# effective scale matches the ACT path's exp(s/8 - 3*ln2) = exp(s/8)*2^-3.
SCH_A = 1024.0 * math.log2(math.e) / 8.0          # 184.664
SCH_B = 12288.0 - 1024.0 * math.log2(1.0406)      # ~12229.2
ACT_BIAS = -3.0 * math.log(2.0)                   # -2.0794

# engine load-balance: which s_k chunks take the DVE Schraudolph exp, and
# which ACT chunks get their mask multiply on GPSIMD instead of DVE
# engine load-balance: which s_k chunks take the DVE Schraudolph exp (the
# rest use ACT exp), and which ACT chunks get their mask multiply on GPSIMD
# instead of DVE.  GPSIMD's ~2.5us op latency would stall the in-order PE
# queue, so GPSIMD-masked chunks are the first NDEFER js and their AV
# matmuls are emitted NDEFER+2 chunks late (PSUM accumulation order is free).
DVE_CHUNKS = frozenset({3, 7, 11, 15})
GPS_MASK_CHUNKS = frozenset()
NDEFER = 0


def _build():
    import concourse.bass as bass
    import concourse.mybir as mybir
    import concourse.tile as tile
    from concourse import bacc
    from concourse.bass import ds

    f32 = mybir.dt.float32
    f16 = mybir.dt.float16
    i16 = mybir.dt.int16
    Alu = mybir.AluOpType
    Act = mybir.ActivationFunctionType

    nc = bacc.Bacc(name="mha8v2")

    xT_d = nc.dram_tensor("xT", [D, S], f16, kind="ExternalInput")
    mask_d = nc.dram_tensor("mask", [S, S], f16, kind="ExternalInput")
    wq_d = nc.dram_tensor("wq", [D, CD], f16, kind="ExternalInput")
    wk_d = nc.dram_tensor("wk", [D, CD], f16, kind="ExternalInput")
    wv_d = nc.dram_tensor("wv", [D, CD], f16, kind="ExternalInput")
    bq_d = nc.dram_tensor("bq", [CD], f32, kind="ExternalInput")
    bk_d = nc.dram_tensor("bk", [CD], f32, kind="ExternalInput")
    bv_d = nc.dram_tensor("bv", [CD], f32, kind="ExternalInput")
    wo_d = nc.dram_tensor("wo", [CD, D], f16, kind="ExternalInput")
    out_d = nc.dram_tensor("out", [S, D], f16, kind="ExternalOutput")

    with tile.TileContext(nc) as tc, ExitStack() as top:
        const = top.enter_context(tc.tile_pool(name="const", bufs=1))

        # HAM warm-up fodder: the PE clock-gate opens after ~3.4us of matmul
        # activity; burn junk matmuls while the input DMAs stream.
        warm_sb = const.tile([128, 512], f16)
        nc.vector.memset(warm_sb, 0.0)

        exp_bias = const.tile([128, 1], f32)
        nc.vector.memset(exp_bias, ACT_BIAS)

        # biases for q/k: [128, 4] -> column m*128+p holds b[m*128+p]
        bq_sb = const.tile([128, NBLK], f32)
        bk_sb = const.tile([128, NBLK], f32)
        nc.sync.dma_start(out=bq_sb, in_=bq_d.rearrange("(m p) -> p m", p=128))
        nc.sync.dma_start(out=bk_sb, in_=bk_d.rearrange("(m p) -> p m", p=128))

        # persistent activation tensors
        qk_pool = top.enter_context(tc.tile_pool(name="qk", bufs=1))
        qT_sb = qk_pool.tile([128, NPAIR, S], f16)   # q^T, head pair-major
        kT_sb = qk_pool.tile([128, NPAIR, S], f16)
        v_pool = top.enter_context(tc.tile_pool(name="vpool", bufs=1))
        v_sb = v_pool.tile([128, SK_CHUNKS, HPC, 66], f16)  # [.., 0:64]=v, 64=ones
        ctx_pool = top.enter_context(tc.tile_pool(name="ctxp", bufs=1))
        ctxT_sb = ctx_pool.tile([128, NPAIR, S], f16)
        mask_pool = top.enter_context(tc.tile_pool(name="maskp", bufs=1))
        mk_sb = mask_pool.tile([128, SK_CHUNKS, S], f16)    # mask[s_k, s_q] 0/1

        nc.vector.memset(v_sb[:, :, :, 64:65], 1.0)

        # ---------------- phase 1: QKV projection -------------------------
        with ExitStack() as ph1:
            wpool = ph1.enter_context(tc.tile_pool(name="wpool", bufs=1))
            xt_pool = ph1.enter_context(tc.tile_pool(name="xt", bufs=1))
            warm_ps_pool = ph1.enter_context(
                tc.tile_pool(name="warmps", bufs=1, space="PSUM")
            )
            prj_ps_pool = ph1.enter_context(
                tc.tile_pool(name="prjps", bufs=4, space="PSUM")
            )

            warm_ps = warm_ps_pool.tile([128, 512], f32)
            for _ in range(16):
                nc.tensor.matmul(
                    warm_ps, lhsT=warm_sb[:, 0:128], rhs=warm_sb,
                    start=True, stop=True,
                )

            # DMA order matters: wq + first xT half gate the first matmuls.
            wq_sb = wpool.tile([128, 8, CD], f16)
            wk_sb = wpool.tile([128, 8, CD], f16)
            wv_sb = wpool.tile([128, 8, CD], f16)
            xT = xt_pool.tile([128, 8, S], f16)
            nc.sync.dma_start(out=wq_sb, in_=wq_d.rearrange("(k p) n -> p k n", p=128))
            nc.sync.dma_start(
                out=xT[:, :, 0:1024],
                in_=xT_d.rearrange("(k p) s -> p k s", p=128)[:, :, 0:1024],
            )
            nc.sync.dma_start(
                out=xT[:, :, 1024:2048],
                in_=xT_d.rearrange("(k p) s -> p k s", p=128)[:, :, 1024:2048],
            )
            nc.sync.dma_start(out=wk_sb, in_=wk_d.rearrange("(k p) n -> p k n", p=128))
            nc.sync.dma_start(out=wv_sb, in_=wv_d.rearrange("(k p) n -> p k n", p=128))

            bv_row = wpool.tile([1, CD], f32)
            nc.sync.dma_start(out=bv_row, in_=bv_d[None, :])
            bv_bc = wpool.tile([128, CD], f32)
            nc.gpsimd.partition_broadcast(bv_bc, bv_row)

            # mask DMAs on the gpsimd queue; needed only at attention start
            for j in range(SK_CHUNKS):
                nc.gpsimd.dma_start(
                    out=mk_sb[:, j, :], in_=mask_d[ds(j * 128, 128), :]
                )

            # qT / kT: [128 (pair-local dh), pair, S]
            for which, w_sb, b_sb, dst in (
                ("q", wq_sb, bq_sb, qT_sb),
                ("k", wk_sb, bk_sb, kT_sb),
            ):
                for m in range(4):
                    for n in range(NBLK):
                        pps = prj_ps_pool.tile([128, 512], f32, tag="pps")
                        for k in range(8):
                            nc.tensor.matmul(
                                pps,
                                lhsT=w_sb[:, k, ds(m * 128, 128)],
                                rhs=xT[:, k, ds(n * 512, 512)],
                                start=(k == 0),
                                stop=(k == 7),
                            )
                        nc.vector.tensor_scalar_add(
                            out=dst[:, m, ds(n * 512, 512)],
                            in0=pps,
                            scalar1=b_sb[:, ds(m, 1)],
                        )

            # v (natural layout + bias; ones column preset)
            for m16 in range(16):
                vps = prj_ps_pool.tile([128, 512], f32, tag="pps")
                for k in range(8):
                    nc.tensor.matmul(
                        vps,
                        lhsT=xT[:, k, ds(m16 * 128, 128)],
                        rhs=wv_sb[:, k, :],
                        start=(k == 0),
                        stop=(k == 7),
                    )
                nc.vector.tensor_tensor(
                    out=v_sb[:, m16, :, 0:64],
                    in0=vps.rearrange("p (h e) -> p h e", h=HPC),
                    in1=bv_bc.rearrange("p (h e) -> p h e", h=HPC),
                    op=Alu.add,
                )

        # ---------------- phase 2: attention ------------------------------
        with ExitStack() as ph2:
            expm_pool = ph2.enter_context(tc.tile_pool(name="expm", bufs=4))
            raw_pool = ph2.enter_context(tc.tile_pool(name="rawp", bufs=4))
            sc_ps_pool = ph2.enter_context(
                tc.tile_pool(name="scps", bufs=3, space="PSUM")
            )
            ctx_ps_pool = ph2.enter_context(
                tc.tile_pool(name="ctxps", bufs=2, space="PSUM")
            )
            rc_pool = ph2.enter_context(tc.tile_pool(name="rcp", bufs=3))

            for qb in range(NQB):          # 512-wide s_q block
                q0 = qb * 512
                for c in range(NPAIR):     # head pair
                    ctx_ps = [
                        ctx_ps_pool.tile(
                            [128, 512], f32, tag="ctxps", name=f"ctxps{a}"
                        )
                        for a in range(2)
                    ]

                    def emit_qk(j):
                        sc = sc_ps_pool.tile([128, 2, 512], f32, tag="scps")
                        # a=0/a=1 hit disjoint PE row groups (base partitions
                        # 0/64) and run concurrently
                        for a in range(2):
                            nc.tensor.matmul(
                                sc[:, a, :],
                                lhsT=kT_sb[ds(a * 64, 64), c, ds(j * 128, 128)],
                                rhs=qT_sb[ds(a * 64, 64), c, ds(q0, 512)],
                                start=True,
                                stop=True,
                            )
                        return sc

                    def emit_ew(j, sc):
                        mslice = mk_sb[:, j, None, ds(q0, 512)].to_broadcast(
                            (128, 2, 512)
                        )
                        expm = expm_pool.tile([128, 2, 512], i16, tag="expm")
                        if j in DVE_CHUNKS:
                            # DVE path: Schraudolph int16 exp, then mask
                            u = raw_pool.tile([128, 2, 512], i16, tag="raw")
                            nc.vector.tensor_scalar(
                                out=u,
                                in0=sc,
                                scalar1=SCH_A,
                                scalar2=SCH_B,
                                op0=Alu.mult,
                                op1=Alu.add,
                            )
                            nc.vector.tensor_tensor(
                                out=expm,
                                in0=u,
                                in1=mslice,
                                op=Alu.mult,
                            )
                        else:
                            # ACT path: exact exp(s/8 - 3ln2), then mask on
                            # DVE or GPSIMD (load-balanced)
                            raw = raw_pool.tile([128, 2, 512], i16, tag="raw")
                            nc.scalar.activation(
                                out=raw.bitcast(f16),
                                in_=sc,
                                func=Act.Exp,
                                scale=0.125,
                                bias=exp_bias,
                            )
                            eng = nc.gpsimd if j in GPS_MASK_CHUNKS else nc.vector
                            eng.tensor_tensor(
                                out=expm.bitcast(f16),
                                in0=raw.bitcast(f16),
                                in1=mslice,
                                op=Alu.mult,
                            )
                        return expm

                    # emission order: AV(NDEFER) first; AV(jd<NDEFER) rides at
                    # jp=jd+NDEFER+2; AV(15) is always emitted last
                    first_av = NDEFER
                    last_av = SK_CHUNKS - 1

                    def emit_av(j, expm):
                        for a in range(2):
                            nc.tensor.matmul(
                                ctx_ps[a][0:65, :],
                                lhsT=v_sb[:, j, c * 2 + a, 0:65],
                                rhs=expm.bitcast(f16)[:, a, :],
                                start=(j == first_av),
                                stop=(j == last_av),
                            )

                    expms = {}
                    pending = None
                    for j in range(SK_CHUNKS):
                        sc = emit_qk(j)
                        if pending is not None:
                            jp = pending[0]
                            expms[jp] = emit_ew(*pending)
                            if jp >= NDEFER:
                                emit_av(jp, expms.pop(jp))
                            # deferred AV of chunk jp-NDEFER-2 rides here
                            jd = jp - NDEFER - 2
                            if 0 <= jd < NDEFER:
                                emit_av(jd, expms.pop(jd))
                        pending = (j, sc)
                    jp = pending[0]
                    expms[jp] = emit_ew(*pending)
                    emit_av(jp, expms.pop(jp))
                    for jd in sorted(expms):
                        emit_av(jd, expms.pop(jd))

                    # evacuate ctx PSUM fast (ctx bufs=2): unnormalized
                    # f16 copy (DVE) + den row copy (ACT), then normalize
                    # in-place in SBUF later (DVE recip, GPSIMD bcast+mult)
                    den2 = rc_pool.tile([1, 2, 512], f32, tag="den")
                    for a in range(2):
                        nc.vector.tensor_copy(
                            out=ctxT_sb[ds(a * 64, 64), c, ds(q0, 512)],
                            in_=ctx_ps[a][0:64, :],
                        )
                        nc.scalar.copy(
                            out=den2[0:1, a, :], in_=ctx_ps[a][64:65, :]
                        )
                    denr = rc_pool.tile([1, 2, 512], f32, tag="denr")
                    nc.vector.reciprocal_approx_fast(out=denr, in_=den2)
                    rbc = rc_pool.tile([128, 2, 512], f32, tag="rbc")
                    nc.gpsimd.partition_broadcast(rbc, denr)
                    for a in range(2):
                        nc.gpsimd.tensor_tensor(
                            out=ctxT_sb[ds(a * 64, 64), c, ds(q0, 512)],
                            in0=ctxT_sb[ds(a * 64, 64), c, ds(q0, 512)],
                            in1=rbc[ds(a * 64, 64), a, :],
                            op=Alu.mult,
                        )

        # ---------------- phase 3: output projection -----------------------
        with ExitStack() as ph3:
            out_ps_pool = ph3.enter_context(
                tc.tile_pool(name="outps", bufs=3, space="PSUM")
            )
            ost_pool = ph3.enter_context(tc.tile_pool(name="ost", bufs=3))
            ph3_const = ph3.enter_context(tc.tile_pool(name="ph3c", bufs=1))

            wo_sb = ph3_const.tile([128, 4, D], f16)
            nc.sync.dma_start(
                out=wo_sb, in_=wo_d.rearrange("(r p) n -> p r n", p=128)
            )
            for m in range(16):
                ops = out_ps_pool.tile([128, D], f32, tag="ops")
                for r in range(4):
                    for n2 in range(2):
                        nc.tensor.matmul(
                            ops[:, ds(n2 * 512, 512)],
                            lhsT=ctxT_sb[:, r, ds(m * 128, 128)],
                            rhs=wo_sb[:, r, ds(n2 * 512, 512)],
                            start=(r == 0),
                            stop=(r == 3),
                        )
                ost = ost_pool.tile([128, D], f16, tag="ost")
                nc.scalar.copy(out=ost, in_=ops)
                nc.sync.dma_start(out=out_d[ds(m * 128, 128), :], in_=ost)

    nc.compile()
    return nc


_NC = None


def _get_nc():
    global _NC
    if _NC is None:
        _NC = _build()
    return _NC


def make_in_maps(inputs):
    x = np.asarray(inputs["x"], dtype=np.float32)
    mask = np.asarray(inputs["mask"])
    w_qkv = np.asarray(inputs["w_qkv"], dtype=np.float32)
    b_qkv = np.asarray(inputs["b_qkv"], dtype=np.float32)
    w_out = np.asarray(inputs["w_out"], dtype=np.float32)

    w16 = w_qkv.astype(np.float16)
    wo16 = w_out.astype(np.float16)

    in_maps = []
    for core in range(NCORES):
        b = core // 2
        h0 = (core % 2) * CD
        in_maps.append(
            {
                "xT": np.ascontiguousarray(x[b].astype(np.float16).T),
                # device wants mask[s_k, s_q] as fp16 0/1
                "mask": np.ascontiguousarray(
                    mask[b, 0].T.astype(np.float16)
                ),
                "wq": np.ascontiguousarray(w16[:, h0 : h0 + CD]),
                "wk": np.ascontiguousarray(w16[:, D + h0 : D + h0 + CD]),
                "wv": np.ascontiguousarray(w16[:, 2 * D + h0 : 2 * D + h0 + CD]),
                "bq": np.ascontiguousarray(b_qkv[h0 : h0 + CD]),
                "bk": np.ascontiguousarray(b_qkv[D + h0 : D + h0 + CD]),
                "bv": np.ascontiguousarray(b_qkv[2 * D + h0 : 2 * D + h0 + CD]),
                "wo": np.ascontiguousarray(wo16[h0 : h0 + CD, :]),
            }
        )
    return in_maps


def gather_out(core_outs, b_out):
    return np.stack(
        [
            core_outs[2 * b].astype(np.float32)
            + core_outs[2 * b + 1].astype(np.float32)
            + b_out
            for b in range(B)
        ],
        axis=0,
    )


def run(inputs, trace=False):
    """Returns (output, BassKernelResults)."""
    from concourse import bass_utils

    nc = _get_nc()
    in_maps = make_in_maps(inputs)
    res = bass_utils.run_bass_kernel_spmd(
        nc, in_maps, core_ids=list(range(NCORES)), trace=trace
    )
    b_out = np.asarray(inputs["b_out"], dtype=np.float32)
    out = gather_out([r["out"] for r in res.results], b_out)
    return out, res


def kernel(**inputs) -> np.ndarray:
    out, _ = run(inputs, trace=False)
    return out
